# revision 5
# baseline (speedup 1.0000x reference)
"""Trainium2 Bass kernel for nn_Attention_basic (B=16, S=4096, d=1 causal attention).

  q = x @ Wq.T + bq ; k = x @ Wk.T + bk ; v = x @ Wv.T + bv          [B, S]
  scores[b,i,j] = q[b,i] * k[b,j]  (causal j <= i), softmax over j
  out[b,i] = sum_j softmax(scores)[b,i,j] * v[b,j]

Two SPMD launches over 8 NeuronCores (no on-device collectives — a
collective's first barrier costs ~70us of launch skew per execution,
more than the host round trip it would save):

Phase A (projections, tensor-parallel over output rows):
  Core c holds rows [512c, 512c+512) of Wq/Wk/Wv (1/8 of the 192 MiB of
  weights — the memory-roofline term) and computes q/k/v[:, 512c:512c+512]
  for all 16 examples. x is stationary in the PE array; weight slices
  stream through as the moving operand. The bias is folded in via an
  appended ones-row of x / bias-row of W.

Phase B (attention, data-parallel over batch):
  Core c handles examples {2c, 2c+1}. For each example, the rank-1 score
  structure lets ScalarE compute P[j, i] = exp(k_j * q_i) directly with
  the activation instruction's per-partition scale (no materialized
  scores matmul), one 128-row j-block at a time over the causal i-range.
  TensorE then accumulates num_i = sum_j P[j,i] v_j and den_i = sum_j
  P[j,i] against a [v | 1] stationary pair, into PSUM over all j-blocks.
  out = num * (1/den). No max-subtraction: max |score| ~ 22 for this
  data distribution (exp <= 4e9, fp32-safe; verified 1.6e-6 vs ref).

The full causal exp work (B*S^2/2 = 134M exps) runs at ScalarE's
1 elem/lane/cycle and is the compute floor (~66us/example-pair/core).
"""

import contextlib
import ctypes
import os
import sys
import types

import numpy as np

N_CORES = 8
B = 16
S = 4096
MSL = S // N_CORES  # 512: per-core slice of the projection output dim
NBLK = 33  # ceil((S+1)/128): 4096 rows of x.T + 1 bias row, padded to 33*128
NPAD = NBLK * 128  # 4224
BPC = B // N_CORES  # 2 examples per core in phase B
NJB = S // 128  # 32 j-blocks per example
NIC = S // 512  # 8 PSUM output chunks of 512

_AXON_SO = "/opt/axon/libaxon_pjrt.so"


def _install_profile_shim():
    """bass_utils' trace path imports antenv.axon_hooks, which this container
    lacks; provide it, backed by the NRT-profile C ABI of the axon PJRT .so."""
    if "antenv.axon_hooks" in sys.modules:
        return

    def _make_hook():
        try:
            lib = ctypes.CDLL(_AXON_SO)
        except OSError:
            return None
        if not hasattr(lib, "axon_start_nrt_profile"):
            return None
        lib.axon_start_nrt_profile.argtypes = [
            ctypes.POINTER(ctypes.c_int64),
            ctypes.c_size_t,
        ]
        lib.axon_start_nrt_profile.restype = ctypes.c_int64
        lib.axon_stop_nrt_profile.argtypes = [ctypes.c_char_p]
        lib.axon_stop_nrt_profile.restype = ctypes.c_int64

        @contextlib.contextmanager
        def _hook(output_dir: str, device_ids):
            import jax

            jax.devices()
            if device_ids:
                ids = (ctypes.c_int64 * len(device_ids))(*device_ids)
                rc = lib.axon_start_nrt_profile(ids, len(device_ids))
            else:
                rc = lib.axon_start_nrt_profile(None, 0)
            if rc != 0:
                raise RuntimeError(f"axon_start_nrt_profile rc={rc}")
            try:
                yield
            finally:
                n = lib.axon_stop_nrt_profile(str(output_dir).encode())
                print(f"ntff profile: {n} file(s) -> {output_dir}", file=sys.stderr)

        return _hook

    mod = types.ModuleType("antenv.axon_hooks")
    hook = _make_hook()
    mod.get_axon_ntff_profile_hook = lambda: hook
    mod.set_axon_ntff_profile_hook = lambda h: None
    sys.modules["antenv.axon_hooks"] = mod


_install_profile_shim()

import concourse.bacc as bacc
import concourse.mybir as mybir
import concourse.tile as tile
from concourse import bass_utils

# the NEFF dirs are throwaway; don't attempt S3 uploads from the container
bass_utils.upload_artifacts = lambda tmpdir: f"local:{tmpdir}"

F32 = mybir.dt.float32

# filled by kernel() when PROFILE is on: {"proj": ns, "attn": ns}
LAST_PROFILE = {}
PROFILE = os.environ.get("BASS_KERNEL_PROFILE", "0") == "1"

_CACHE = {}


def _build_proj():
    """Phase A: per-core q/k/v projection slices.

    Inputs (pre-tiled host-side so every DMA is contiguous per partition):
      xt        [128, 33*16]   x.T (+ones row, zero pad) tiled (a p) b -> p (a b)
      wq/wk/wv  [128, 33*512]  W.T[:, mslice] (+bias row) tiled (a p) m -> p (a m)
    Outputs: oq/ok/ov [16, 512]
    """
    nc = bacc.Bacc(
        "TRN2", target_bir_lowering=False, debug=False, num_devices=N_CORES
    )
    xt = nc.dram_tensor("xt", [128, NBLK * 16], F32, kind="ExternalInput").ap()
    ws = [
        nc.dram_tensor(f"w{n}", [128, NBLK * MSL], F32, kind="ExternalInput").ap()
        for n in "qkv"
    ]
    outs = [
        nc.dram_tensor(f"o{n}", [B, MSL], F32, kind="ExternalOutput").ap()
        for n in "qkv"
    ]

    with tile.TileContext(nc) as tc:
        with (
            tc.tile_pool(name="xp", bufs=1) as xp,
            tc.tile_pool(name="wp", bufs=4) as wp,
            tc.tile_pool(name="op", bufs=3) as op,
            tc.tile_pool(name="ps", bufs=1, space="PSUM") as pp,
        ):
            x_sb = xp.tile([128, NBLK * 16], F32)
            nc.sync.dma_start(x_sb[:], xt[:])
            ST = 8  # a-blocks per DMA supertile (2 MiB per transfer)
            for pi in range(3):
                ps = pp.tile([B, MSL], F32, tag=f"acc{pi}")
                for a0 in range(0, NBLK, ST):
                    na = min(ST, NBLK - a0)
                    wt = wp.tile([128, ST * MSL], F32, tag="w")
                    nc.sync.dma_start(
                        wt[:, : na * MSL], ws[pi][:, a0 * MSL : (a0 + na) * MSL]
                    )
                    for aa in range(na):
                        a = a0 + aa
                        nc.tensor.matmul(
                            ps[:],
                            x_sb[:, a * 16 : (a + 1) * 16],
                            wt[:, aa * MSL : (aa + 1) * MSL],
                            start=(a == 0),
                            stop=(a == NBLK - 1),
                        )
                osb = op.tile([B, MSL], F32, tag="o")
                nc.vector.tensor_copy(osb[:], ps[:])
                nc.sync.dma_start(outs[pi][:], osb[:])
    nc.compile()
    return nc


def _build_attn():
    """Phase B: causal d=1 attention for 2 examples per core.

    Inputs:
      qb   [2, 128, S]  q broadcast across partitions (host-side)
      kt   [2, 128, 32] k tiled j-major: kt[b, p, a] = k[b, a*128+p]
      w2   [2, 128, 64] interleaved [v | 1] stationary pairs:
                        w2[b, p, 2a] = v[b, a*128+p], w2[b, p, 2a+1] = 1
      mask [128, 128]   mask[p, i] = 1 if p <= i else 0 (causal, diag block)
    Output: out [2, S]
    """
    nc = bacc.Bacc(
        "TRN2", target_bir_lowering=False, debug=False, num_devices=N_CORES
    )
    qb = nc.dram_tensor("qb", [BPC, 128, S], F32, kind="ExternalInput").ap()
    kt = nc.dram_tensor("kt", [BPC, 128, NJB], F32, kind="ExternalInput").ap()
    w2 = nc.dram_tensor("w2", [BPC, 128, 2 * NJB], F32, kind="ExternalInput").ap()
    mask = nc.dram_tensor("mask", [128, 128], F32, kind="ExternalInput").ap()
    out = nc.dram_tensor("out", [BPC, S], F32, kind="ExternalOutput").ap()

    with tile.TileContext(nc) as tc:
        with (
            tc.tile_pool(name="cst", bufs=1) as cst,
            tc.tile_pool(name="qp", bufs=2) as qp,
            tc.tile_pool(name="kp", bufs=2) as kp,
            tc.tile_pool(name="pp", bufs=3) as ppool,
            tc.tile_pool(name="ep", bufs=1) as ep,
            tc.tile_pool(name="ps", bufs=1, space="PSUM") as psp,
        ):
            mask_sb = cst.tile([128, 128], F32)
            nc.sync.dma_start(mask_sb[:], mask[:])
            for b in range(BPC):
                qb_sb = qp.tile([128, S], F32, tag="qb")
                nc.sync.dma_start(qb_sb[:], qb[b])
                k_sb = kp.tile([128, NJB], F32, tag="k")
                nc.sync.dma_start(k_sb[:], kt[b])
                w2_sb = kp.tile([128, 2 * NJB], F32, tag="w2")
                nc.sync.dma_start(w2_sb[:], w2[b])
                acc = psp.tile([2, S], F32, tag="acc")
                for jb in range(NJB):
                    F = S - 128 * jb
                    P = ppool.tile([128, S], F32, tag="P")
                    nc.scalar.activation(
                        P[:, :F],
                        qb_sb[:, 128 * jb :],
                        mybir.ActivationFunctionType.Exp,
                        scale=k_sb[:, jb : jb + 1],
                    )
                    # causal mask inside the diagonal 128x128 block
                    nc.vector.tensor_mul(P[:, 0:128], P[:, 0:128], mask_sb[:])
                    for ic in range(jb // 4, NIC):
                        g0 = max(512 * ic, 128 * jb)
                        n = 512 * (ic + 1) - g0
                        nc.tensor.matmul(
                            acc[0:2, g0 : g0 + n],
                            w2_sb[:, 2 * jb : 2 * jb + 2],
                            P[:, g0 - 128 * jb : g0 - 128 * jb + n],
                            start=(jb == 0),
                            stop=(jb == min(4 * ic + 3, NJB - 1)),
                        )
                nd_sb = ep.tile([2, S], F32, tag="nd")
                nc.vector.tensor_copy(nd_sb[:], acc[0:2, :])
                # den lives on partition 1; DVE can't shift partitions, DMA can
                den_sb = ep.tile([1, S], F32, tag="den")
                nc.sync.dma_start(den_sb[:], nd_sb[1:2, :])
                nc.vector.reciprocal_approx_fast(den_sb[:], den_sb[:])
                nc.vector.tensor_mul(nd_sb[0:1, :], nd_sb[0:1, :], den_sb[:])
                nc.sync.dma_start(out[b : b + 1, :], nd_sb[0:1, :])
    nc.compile()
    return nc


def _get(name, builder):
    if name not in _CACHE:
        _CACHE[name] = builder()
    return _CACHE[name]


def _run(nc, in_maps, tag):
    res = bass_utils.run_bass_kernel_spmd(
        nc, in_maps, core_ids=list(range(N_CORES)), trace=PROFILE
    )
    if PROFILE:
        LAST_PROFILE[tag] = res.exec_time_ns
        LAST_PROFILE[f"{tag}_trace"] = res.instructions_and_trace
    return res.results


def kernel(x, Wq, bq, Wk, bk, Wv, bv):
    x = np.ascontiguousarray(np.asarray(x, dtype=np.float32))
    Ws = [np.asarray(W, dtype=np.float32) for W in (Wq, Wk, Wv)]
    bs = [np.asarray(bb, dtype=np.float32) for bb in (bq, bk, bv)]

    # ---- phase A host prep ----
    xta = np.zeros((NPAD, B), np.float32)
    xta[:S] = x.T
    xta[S, :] = 1.0  # ones row folds the bias into the matmul
    xt_tiled = np.ascontiguousarray(
        xta.reshape(NBLK, 128, B).transpose(1, 0, 2).reshape(128, NBLK * B)
    )
    in_maps_a = []
    for c in range(N_CORES):
        m = {"xt": xt_tiled}
        sl = slice(c * MSL, (c + 1) * MSL)
        for name, W, bias in zip("qkv", Ws, bs):
            wa = np.zeros((NPAD, MSL), np.float32)
            wa[:S] = W[sl].T
            wa[S] = bias[sl]
            m[f"w{name}"] = np.ascontiguousarray(
                wa.reshape(NBLK, 128, MSL).transpose(1, 0, 2).reshape(128, NBLK * MSL)
            )
        in_maps_a.append(m)

    res_a = _run(_get("proj", _build_proj), in_maps_a, "proj")
    q = np.concatenate([res_a[c]["oq"] for c in range(N_CORES)], axis=1)
    k = np.concatenate([res_a[c]["ok"] for c in range(N_CORES)], axis=1)
    v = np.concatenate([res_a[c]["ov"] for c in range(N_CORES)], axis=1)

    # ---- phase B host prep ----
    mask = np.ascontiguousarray(np.triu(np.ones((128, 128), np.float32)))
    in_maps_b = []
    for c in range(N_CORES):
        ex = slice(BPC * c, BPC * (c + 1))
        qb = np.ascontiguousarray(
            np.broadcast_to(q[ex][:, None, :], (BPC, 128, S))
        )
        ktc = np.ascontiguousarray(
            k[ex].reshape(BPC, NJB, 128).transpose(0, 2, 1)
        )
        vtc = v[ex].reshape(BPC, NJB, 128).transpose(0, 2, 1)
        w2 = np.empty((BPC, 128, 2 * NJB), np.float32)
        w2[:, :, 0::2] = vtc
        w2[:, :, 1::2] = 1.0
        in_maps_b.append({"qb": qb, "kt": ktc, "w2": w2, "mask": mask})

    res_b = _run(_get("attn", _build_attn), in_maps_b, "attn")
    out = np.concatenate([res_b[c]["out"] for c in range(N_CORES)], axis=0)
    return out


# revision 9
# speedup vs baseline: 1.7517x; 1.7517x over previous
"""Trainium2 Bass kernel for nn_Attention_basic (B=16, S=4096, d=1 causal attention).

  q = x @ Wq.T + bq ; k = x @ Wk.T + bk ; v = x @ Wv.T + bv          [B, S]
  scores[b,i,j] = q[b,i] * k[b,j]  (causal j <= i), softmax over j
  out[b,i] = sum_j softmax(scores)[b,i,j] * v[b,j]

Two SPMD launches over 8 NeuronCores (no on-device collectives — a
collective's first barrier costs ~70us of launch skew per execution,
more than the host round trip it would save):

Phase A (projections, tensor-parallel over output rows):
  Core c holds rows [512c, 512c+512) of Wq/Wk/Wv (1/8 of the 192 MiB of
  weights — the memory-roofline term) and computes q/k/v[:, 512c:512c+512]
  for all 16 examples. x is stationary in the PE array; weight slices
  stream through as the moving operand. The bias is folded in via an
  appended ones-row of x / bias-row of W.

Phase B (attention, data-parallel over batch):
  Core c handles examples {2c, 2c+1}. For each example, the rank-1 score
  structure lets ScalarE compute P[j, i] = exp(k_j * q_i) directly with
  the activation instruction's per-partition scale (no materialized
  scores matmul), one 128-row j-block at a time over the causal i-range.
  TensorE then accumulates num_i = sum_j P[j,i] v_j and den_i = sum_j
  P[j,i] against a [v | 1] stationary pair, into PSUM over all j-blocks.
  out = num * (1/den). No max-subtraction: max |score| ~ 22 for this
  data distribution (exp <= 4e9, fp32-safe; verified 1.6e-6 vs ref).

The full causal exp work (B*S^2/2 = 134M exps) runs at ScalarE's
1 elem/lane/cycle and is the compute floor (~66us/example-pair/core).
"""

import contextlib
import ctypes
import os
import sys
import types

import numpy as np
import ml_dtypes

N_CORES = 8
B = 16
S = 4096
MSL = S // N_CORES  # 512: per-core slice of the projection output dim
NBLK = 33  # ceil((S+1)/128): 4096 rows of x.T + 1 bias row, padded to 33*128
NPAD = NBLK * 128  # 4224
BPC = B // N_CORES  # 2 examples per core in phase B
NJB = S // 128  # 32 j-blocks per example
NIC = S // 512  # 8 PSUM output chunks of 512

_AXON_SO = "/opt/axon/libaxon_pjrt.so"


def _install_profile_shim():
    """bass_utils' trace path imports antenv.axon_hooks, which this container
    lacks; provide it, backed by the NRT-profile C ABI of the axon PJRT .so."""
    if "antenv.axon_hooks" in sys.modules:
        return

    def _make_hook():
        try:
            lib = ctypes.CDLL(_AXON_SO)
        except OSError:
            return None
        if not hasattr(lib, "axon_start_nrt_profile"):
            return None
        lib.axon_start_nrt_profile.argtypes = [
            ctypes.POINTER(ctypes.c_int64),
            ctypes.c_size_t,
        ]
        lib.axon_start_nrt_profile.restype = ctypes.c_int64
        lib.axon_stop_nrt_profile.argtypes = [ctypes.c_char_p]
        lib.axon_stop_nrt_profile.restype = ctypes.c_int64

        @contextlib.contextmanager
        def _hook(output_dir: str, device_ids):
            import jax

            jax.devices()
            if device_ids:
                ids = (ctypes.c_int64 * len(device_ids))(*device_ids)
                rc = lib.axon_start_nrt_profile(ids, len(device_ids))
            else:
                rc = lib.axon_start_nrt_profile(None, 0)
            if rc != 0:
                raise RuntimeError(f"axon_start_nrt_profile rc={rc}")
            try:
                yield
            finally:
                n = lib.axon_stop_nrt_profile(str(output_dir).encode())
                print(f"ntff profile: {n} file(s) -> {output_dir}", file=sys.stderr)

        return _hook

    mod = types.ModuleType("antenv.axon_hooks")
    hook = _make_hook()
    mod.get_axon_ntff_profile_hook = lambda: hook
    mod.set_axon_ntff_profile_hook = lambda h: None
    sys.modules["antenv.axon_hooks"] = mod


_install_profile_shim()

import concourse.bacc as bacc
import concourse.mybir as mybir
import concourse.tile as tile
from concourse import bass_utils

# the NEFF dirs are throwaway; don't attempt S3 uploads from the container
bass_utils.upload_artifacts = lambda tmpdir: f"local:{tmpdir}"

F32 = mybir.dt.float32
F16 = mybir.dt.float16
BF16 = mybir.dt.bfloat16

# filled by kernel() when PROFILE is on: {"proj": ns, "attn": ns}
LAST_PROFILE = {}
PROFILE = os.environ.get("BASS_KERNEL_PROFILE", "0") == "1"

_CACHE = {}


def _build_proj():
    """Phase A: per-core q/k/v projection slices.

    Inputs (pre-tiled host-side so every DMA is contiguous per partition):
      xt        [128, 33*16]   x.T (+ones row, zero pad) tiled (a p) b -> p (a b)
      wq/wk/wv  [128, 33*512]  W.T[:, mslice] (+bias row) tiled (a p) m -> p (a m)
    Outputs: oq/ok/ov [16, 512]
    """
    nc = bacc.Bacc(
        "TRN2", target_bir_lowering=False, debug=False, num_devices=N_CORES
    )
    xt = nc.dram_tensor("xt", [128, NBLK * 16], F16, kind="ExternalInput").ap()
    ws = [
        nc.dram_tensor(f"w{n}", [128, NBLK * MSL], F16, kind="ExternalInput").ap()
        for n in "qkv"
    ]
    outs = [
        nc.dram_tensor(f"o{n}", [B, MSL], F32, kind="ExternalOutput").ap()
        for n in "qkv"
    ]

    with tile.TileContext(nc) as tc:
        with (
            tc.tile_pool(name="xp", bufs=1) as xp,
            tc.tile_pool(name="wp", bufs=4) as wp,
            tc.tile_pool(name="op", bufs=3) as op,
            tc.tile_pool(name="ps", bufs=1, space="PSUM") as pp,
        ):
            x_sb = xp.tile([128, NBLK * 16], F16)
            nc.sync.dma_start(x_sb[:], xt[:])
            ST = 16  # a-blocks per DMA supertile (2 MiB fp16 per transfer)
            for pi in range(3):
                ps = pp.tile([B, MSL], F32, tag=f"acc{pi}")
                for a0 in range(0, NBLK, ST):
                    na = min(ST, NBLK - a0)
                    wt = wp.tile([128, ST * MSL], F16, tag="w")
                    nc.sync.dma_start(
                        wt[:, : na * MSL], ws[pi][:, a0 * MSL : (a0 + na) * MSL]
                    )
                    for aa in range(na):
                        a = a0 + aa
                        nc.tensor.matmul(
                            ps[:],
                            x_sb[:, a * 16 : (a + 1) * 16],
                            wt[:, aa * MSL : (aa + 1) * MSL],
                            start=(a == 0),
                            stop=(a == NBLK - 1),
                        )
                osb = op.tile([B, MSL], F32, tag="o")
                nc.vector.tensor_copy(osb[:], ps[:])
                nc.sync.dma_start(outs[pi][:], osb[:])
    nc.compile()
    return nc


def _build_attn():
    """Phase B: causal d=1 attention for 2 examples per core.

    Inputs:
      qb   [2, 128, S]  q broadcast across partitions (host-side)
      kt   [2, 128, 32] k tiled j-major: kt[b, p, a] = k[b, a*128+p]
      w2   [2, 128, 64] interleaved [v | 1] stationary pairs:
                        w2[b, p, 2a] = v[b, a*128+p], w2[b, p, 2a+1] = 1
      mask [128, 128]   mask[p, i] = 1 if p <= i else 0 (causal, diag block)
    Output: out [2, S]
    """
    nc = bacc.Bacc(
        "TRN2", target_bir_lowering=False, debug=False, num_devices=N_CORES
    )
    qb = nc.dram_tensor("qb", [BPC, 128, S], F32, kind="ExternalInput").ap()
    kt = nc.dram_tensor("kt", [BPC, 128, NJB], F32, kind="ExternalInput").ap()
    w2 = nc.dram_tensor("w2", [BPC, 128, 2 * NJB], BF16, kind="ExternalInput").ap()
    mask = nc.dram_tensor("mask", [128, 128], BF16, kind="ExternalInput").ap()
    out = nc.dram_tensor("out", [BPC, S], F32, kind="ExternalOutput").ap()

    with tile.TileContext(nc) as tc:
        with (
            tc.tile_pool(name="cst", bufs=1) as cst,
            tc.tile_pool(name="qp", bufs=2) as qp,
            tc.tile_pool(name="kp", bufs=2) as kp,
            tc.tile_pool(name="pp", bufs=4) as ppool,
            tc.tile_pool(name="ep", bufs=1) as ep,
            tc.tile_pool(name="ps", bufs=1, space="PSUM") as psp,
        ):
            mask_sb = cst.tile([128, 128], BF16)
            nc.sync.dma_start(mask_sb[:], mask[:])
            for b in range(BPC):
                qb_sb = qp.tile([128, S], F32, tag="qb")
                nc.sync.dma_start(qb_sb[:], qb[b])
                k_sb = kp.tile([128, NJB], F32, tag="k")
                nc.sync.dma_start(k_sb[:], kt[b])
                w2_sb = kp.tile([128, 2 * NJB], BF16, tag="w2")
                nc.sync.dma_start(w2_sb[:], w2[b])
                acc = psp.tile([2, S], F32, tag="acc")
                for jb in range(NJB):
                    F = S - 128 * jb
                    P = ppool.tile([128, S], BF16, tag="P")
                    nc.scalar.activation(
                        P[:, :F],
                        qb_sb[:, 128 * jb :],
                        mybir.ActivationFunctionType.Exp,
                        scale=k_sb[:, jb : jb + 1],
                    )
                    # causal mask inside the diagonal 128x128 block
                    nc.vector.tensor_mul(P[:, 0:128], P[:, 0:128], mask_sb[:])
                    for ic in range(jb // 4, NIC):
                        g0 = max(512 * ic, 128 * jb)
                        n = 512 * (ic + 1) - g0
                        nc.tensor.matmul(
                            acc[0:2, g0 : g0 + n],
                            w2_sb[:, 2 * jb : 2 * jb + 2],
                            P[:, g0 - 128 * jb : g0 - 128 * jb + n],
                            start=(jb == 0),
                            stop=(jb == min(4 * ic + 3, NJB - 1)),
                        )
                nd_sb = ep.tile([2, S], F32, tag="nd")
                nc.vector.tensor_copy(nd_sb[:], acc[0:2, :])
                # den lives on partition 1; DVE can't shift partitions, DMA can
                den_sb = ep.tile([1, S], F32, tag="den")
                nc.sync.dma_start(den_sb[:], nd_sb[1:2, :])
                nc.vector.reciprocal_approx_fast(den_sb[:], den_sb[:])
                nc.vector.tensor_mul(nd_sb[0:1, :], nd_sb[0:1, :], den_sb[:])
                nc.sync.dma_start(out[b : b + 1, :], nd_sb[0:1, :])
    nc.compile()
    return nc


def _get(name, builder):
    if name not in _CACHE:
        _CACHE[name] = builder()
    return _CACHE[name]


def _run(nc, in_maps, tag):
    res = bass_utils.run_bass_kernel_spmd(
        nc, in_maps, core_ids=list(range(N_CORES)), trace=PROFILE
    )
    if PROFILE:
        LAST_PROFILE[tag] = res.exec_time_ns
        LAST_PROFILE[f"{tag}_trace"] = res.instructions_and_trace
    return res.results


def kernel(x, Wq, bq, Wk, bk, Wv, bv):
    x = np.ascontiguousarray(np.asarray(x, dtype=np.float32))
    Ws = [np.asarray(W, dtype=np.float32) for W in (Wq, Wk, Wv)]
    bs = [np.asarray(bb, dtype=np.float32) for bb in (bq, bk, bv)]

    # ---- phase A host prep ----
    xta = np.zeros((NPAD, B), np.float32)
    xta[:S] = x.T
    xta[S, :] = 1.0  # ones row folds the bias into the matmul
    xt_tiled = np.ascontiguousarray(
        xta.reshape(NBLK, 128, B).transpose(1, 0, 2).reshape(128, NBLK * B)
    ).astype(np.float16)
    in_maps_a = []
    for c in range(N_CORES):
        m = {"xt": xt_tiled}
        sl = slice(c * MSL, (c + 1) * MSL)
        for name, W, bias in zip("qkv", Ws, bs):
            wa = np.zeros((NPAD, MSL), np.float32)
            wa[:S] = W[sl].T
            wa[S] = bias[sl]
            m[f"w{name}"] = np.ascontiguousarray(
                wa.reshape(NBLK, 128, MSL).transpose(1, 0, 2).reshape(128, NBLK * MSL)
            ).astype(np.float16)
        in_maps_a.append(m)

    res_a = _run(_get("proj", _build_proj), in_maps_a, "proj")
    q = np.concatenate([res_a[c]["oq"] for c in range(N_CORES)], axis=1)
    k = np.concatenate([res_a[c]["ok"] for c in range(N_CORES)], axis=1)
    v = np.concatenate([res_a[c]["ov"] for c in range(N_CORES)], axis=1)

    # ---- phase B host prep ----
    mask = np.ascontiguousarray(
        np.triu(np.ones((128, 128))).astype(ml_dtypes.bfloat16)
    )
    in_maps_b = []
    for c in range(N_CORES):
        ex = slice(BPC * c, BPC * (c + 1))
        qb = np.ascontiguousarray(
            np.broadcast_to(q[ex][:, None, :], (BPC, 128, S))
        )
        ktc = np.ascontiguousarray(
            k[ex].reshape(BPC, NJB, 128).transpose(0, 2, 1)
        )
        vtc = v[ex].reshape(BPC, NJB, 128).transpose(0, 2, 1)
        w2 = np.empty((BPC, 128, 2 * NJB), np.float32)
        w2[:, :, 0::2] = vtc
        w2[:, :, 1::2] = 1.0
        w2 = w2.astype(ml_dtypes.bfloat16)
        in_maps_b.append({"qb": qb, "kt": ktc, "w2": w2, "mask": mask})

    res_b = _run(_get("attn", _build_attn), in_maps_b, "attn")
    out = np.concatenate([res_b[c]["out"] for c in range(N_CORES)], axis=0)
    return out


# revision 11
# speedup vs baseline: 1.7598x; 1.0046x over previous
"""Trainium2 Bass kernel for nn_Attention_basic (B=16, S=4096, d=1 causal attention).

  q = x @ Wq.T + bq ; k = x @ Wk.T + bk ; v = x @ Wv.T + bv          [B, S]
  scores[b,i,j] = q[b,i] * k[b,j]  (causal j <= i), softmax over j
  out[b,i] = sum_j softmax(scores)[b,i,j] * v[b,j]

Two SPMD launches over 8 NeuronCores (no on-device collectives — a
collective's first barrier costs ~70us of launch skew per execution,
more than the host round trip it would save):

Phase A (projections, tensor-parallel over output rows):
  Core c holds rows [512c, 512c+512) of Wq/Wk/Wv (1/8 of the 192 MiB of
  weights — the memory-roofline term) and computes q/k/v[:, 512c:512c+512]
  for all 16 examples. x is stationary in the PE array; weight slices
  stream through as the moving operand. The bias is folded in via an
  appended ones-row of x / bias-row of W.

Phase B (attention, data-parallel over batch):
  Core c handles examples {2c, 2c+1}. For each example, the rank-1 score
  structure lets ScalarE compute P[j, i] = exp(k_j * q_i) directly with
  the activation instruction's per-partition scale (no materialized
  scores matmul), one 128-row j-block at a time over the causal i-range.
  TensorE then accumulates num_i = sum_j P[j,i] v_j and den_i = sum_j
  P[j,i] against a [v | 1] stationary pair, into PSUM over all j-blocks.
  out = num * (1/den). No max-subtraction: max |score| ~ 22 for this
  data distribution (exp <= 4e9, fp32-safe; verified 1.6e-6 vs ref).

The full causal exp work (B*S^2/2 = 134M exps) runs at ScalarE's
1 elem/lane/cycle and is the compute floor (~66us/example-pair/core).
"""

import contextlib
import ctypes
import os
import sys
import types

import numpy as np
import ml_dtypes

N_CORES = 8
B = 16
S = 4096
MSL = S // N_CORES  # 512: per-core slice of the projection output dim
NBLK = 33  # ceil((S+1)/128): 4096 rows of x.T + 1 bias row, padded to 33*128
NPAD = NBLK * 128  # 4224
BPC = B // N_CORES  # 2 examples per core in phase B
NJB = S // 128  # 32 j-blocks per example
NIC = S // 512  # 8 PSUM output chunks of 512

_AXON_SO = "/opt/axon/libaxon_pjrt.so"


def _install_profile_shim():
    """bass_utils' trace path imports antenv.axon_hooks, which this container
    lacks; provide it, backed by the NRT-profile C ABI of the axon PJRT .so."""
    if "antenv.axon_hooks" in sys.modules:
        return

    def _make_hook():
        try:
            lib = ctypes.CDLL(_AXON_SO)
        except OSError:
            return None
        if not hasattr(lib, "axon_start_nrt_profile"):
            return None
        lib.axon_start_nrt_profile.argtypes = [
            ctypes.POINTER(ctypes.c_int64),
            ctypes.c_size_t,
        ]
        lib.axon_start_nrt_profile.restype = ctypes.c_int64
        lib.axon_stop_nrt_profile.argtypes = [ctypes.c_char_p]
        lib.axon_stop_nrt_profile.restype = ctypes.c_int64

        @contextlib.contextmanager
        def _hook(output_dir: str, device_ids):
            import jax

            jax.devices()
            if device_ids:
                ids = (ctypes.c_int64 * len(device_ids))(*device_ids)
                rc = lib.axon_start_nrt_profile(ids, len(device_ids))
            else:
                rc = lib.axon_start_nrt_profile(None, 0)
            if rc != 0:
                raise RuntimeError(f"axon_start_nrt_profile rc={rc}")
            try:
                yield
            finally:
                n = lib.axon_stop_nrt_profile(str(output_dir).encode())
                print(f"ntff profile: {n} file(s) -> {output_dir}", file=sys.stderr)

        return _hook

    mod = types.ModuleType("antenv.axon_hooks")
    hook = _make_hook()
    mod.get_axon_ntff_profile_hook = lambda: hook
    mod.set_axon_ntff_profile_hook = lambda h: None
    sys.modules["antenv.axon_hooks"] = mod


_install_profile_shim()

import concourse.bacc as bacc
import concourse.mybir as mybir
import concourse.tile as tile
from concourse import bass_utils

# the NEFF dirs are throwaway; don't attempt S3 uploads from the container
bass_utils.upload_artifacts = lambda tmpdir: f"local:{tmpdir}"

F32 = mybir.dt.float32
F16 = mybir.dt.float16
BF16 = mybir.dt.bfloat16

# filled by kernel() when PROFILE is on: {"proj": ns, "attn": ns}
LAST_PROFILE = {}
PROFILE = os.environ.get("BASS_KERNEL_PROFILE", "0") == "1"

_CACHE = {}


def _build_proj():
    """Phase A: per-core q/k/v projection slices.

    Inputs (pre-tiled host-side so every DMA is contiguous per partition):
      xt        [128, 33*16]   x.T (+ones row, zero pad) tiled (a p) b -> p (a b)
      wq/wk/wv  [128, 33*512]  W.T[:, mslice] (+bias row) tiled (a p) m -> p (a m)
    Outputs: oq/ok/ov [16, 512]
    """
    nc = bacc.Bacc(
        "TRN2", target_bir_lowering=False, debug=False, num_devices=N_CORES
    )
    xt = nc.dram_tensor("xt", [128, NBLK * 16], F16, kind="ExternalInput").ap()
    ws = [
        nc.dram_tensor(f"w{n}", [128, NBLK * MSL], F16, kind="ExternalInput").ap()
        for n in "qkv"
    ]
    outs = [
        nc.dram_tensor(f"o{n}", [B, MSL], F32, kind="ExternalOutput").ap()
        for n in "qkv"
    ]

    with tile.TileContext(nc) as tc:
        with (
            tc.tile_pool(name="xp", bufs=1) as xp,
            tc.tile_pool(name="wp", bufs=6) as wp,
            tc.tile_pool(name="op", bufs=3) as op,
            tc.tile_pool(name="ps", bufs=1, space="PSUM") as pp,
        ):
            x_sb = xp.tile([128, NBLK * 16], F16)
            nc.sync.dma_start(x_sb[:], xt[:])
            ST = 16  # a-blocks per DMA supertile (2 MiB fp16 per transfer)
            for pi in range(3):
                ps = pp.tile([B, MSL], F32, tag=f"acc{pi}")
                for a0 in range(0, NBLK, ST):
                    na = min(ST, NBLK - a0)
                    wt = wp.tile([128, ST * MSL], F16, tag="w")
                    nc.sync.dma_start(
                        wt[:, : na * MSL], ws[pi][:, a0 * MSL : (a0 + na) * MSL]
                    )
                    for aa in range(na):
                        a = a0 + aa
                        nc.tensor.matmul(
                            ps[:],
                            x_sb[:, a * 16 : (a + 1) * 16],
                            wt[:, aa * MSL : (aa + 1) * MSL],
                            start=(a == 0),
                            stop=(a == NBLK - 1),
                        )
                osb = op.tile([B, MSL], F32, tag="o")
                nc.vector.tensor_copy(osb[:], ps[:])
                nc.sync.dma_start(outs[pi][:], osb[:])
    nc.compile()
    return nc


def _build_attn():
    """Phase B: causal d=1 attention for 2 examples per core.

    Inputs:
      qb   [2, 128, S]  q broadcast across partitions (host-side)
      kt   [2, 128, 32] k tiled j-major: kt[b, p, a] = k[b, a*128+p]
      w2   [2, 128, 64] interleaved [v | 1] stationary pairs:
                        w2[b, p, 2a] = v[b, a*128+p], w2[b, p, 2a+1] = 1
      mask [128, 128]   mask[p, i] = 1 if p <= i else 0 (causal, diag block)
    Output: out [2, S]
    """
    nc = bacc.Bacc(
        "TRN2", target_bir_lowering=False, debug=False, num_devices=N_CORES
    )
    qb = nc.dram_tensor("qb", [BPC, 128, S], F32, kind="ExternalInput").ap()
    kt = nc.dram_tensor("kt", [BPC, 128, NJB], F32, kind="ExternalInput").ap()
    w2 = nc.dram_tensor("w2", [BPC, 128, 2 * NJB], BF16, kind="ExternalInput").ap()
    mask = nc.dram_tensor("mask", [128, 128], BF16, kind="ExternalInput").ap()
    out = nc.dram_tensor("out", [BPC, S], F32, kind="ExternalOutput").ap()

    with tile.TileContext(nc) as tc:
        with (
            tc.tile_pool(name="cst", bufs=1) as cst,
            tc.tile_pool(name="qp", bufs=2) as qp,
            tc.tile_pool(name="kp", bufs=2) as kp,
            tc.tile_pool(name="pp", bufs=6) as ppool,
            tc.tile_pool(name="ep", bufs=2) as ep,
            tc.tile_pool(name="ps", bufs=1, space="PSUM") as psp,
        ):
            mask_sb = cst.tile([128, 128], BF16)
            nc.sync.dma_start(mask_sb[:], mask[:])
            for b in range(BPC):
                qb_sb = qp.tile([128, S], F32, tag="qb")
                nc.sync.dma_start(qb_sb[:], qb[b])
                k_sb = kp.tile([128, NJB], F32, tag="k")
                nc.sync.dma_start(k_sb[:], kt[b])
                w2_sb = kp.tile([128, 2 * NJB], BF16, tag="w2")
                nc.sync.dma_start(w2_sb[:], w2[b])
                acc = psp.tile([2, S], F32, tag="acc")
                for jb in range(NJB):
                    F = S - 128 * jb
                    P = ppool.tile([128, S], BF16, tag="P")
                    nc.scalar.activation(
                        P[:, :F],
                        qb_sb[:, 128 * jb :],
                        mybir.ActivationFunctionType.Exp,
                        scale=k_sb[:, jb : jb + 1],
                    )
                    # causal mask inside the diagonal 128x128 block
                    nc.vector.tensor_mul(P[:, 0:128], P[:, 0:128], mask_sb[:])
                    for ic in range(jb // 4, NIC):
                        g0 = max(512 * ic, 128 * jb)
                        n = 512 * (ic + 1) - g0
                        nc.tensor.matmul(
                            acc[0:2, g0 : g0 + n],
                            w2_sb[:, 2 * jb : 2 * jb + 2],
                            P[:, g0 - 128 * jb : g0 - 128 * jb + n],
                            start=(jb == 0),
                            stop=(jb == min(4 * ic + 3, NJB - 1)),
                        )
                    # epilogue per finished 512-chunk, overlapped with the
                    # main loop (chunk ic is complete after jb = 4ic+3)
                    for ic in range(NIC):
                        if min(4 * ic + 3, NJB - 1) != jb:
                            continue
                        g0 = 512 * ic
                        nd = ep.tile([2, 512], F32, tag="nd")
                        nc.vector.tensor_copy(nd[:], acc[0:2, g0 : g0 + 512])
                        # den lives on partition 1; DVE can't shift
                        # partitions, DMA can
                        den = ep.tile([1, 512], F32, tag="den")
                        nc.sync.dma_start(den[:], nd[1:2, :])
                        nc.vector.reciprocal_approx_fast(den[:], den[:])
                        nc.vector.tensor_mul(nd[0:1, :], nd[0:1, :], den[:])
                        nc.sync.dma_start(
                            out[b : b + 1, g0 : g0 + 512], nd[0:1, :]
                        )
    nc.compile()
    return nc


def _get(name, builder):
    if name not in _CACHE:
        _CACHE[name] = builder()
    return _CACHE[name]


def _run(nc, in_maps, tag):
    res = bass_utils.run_bass_kernel_spmd(
        nc, in_maps, core_ids=list(range(N_CORES)), trace=PROFILE
    )
    if PROFILE:
        LAST_PROFILE[tag] = res.exec_time_ns
        LAST_PROFILE[f"{tag}_trace"] = res.instructions_and_trace
    return res.results


def kernel(x, Wq, bq, Wk, bk, Wv, bv):
    x = np.ascontiguousarray(np.asarray(x, dtype=np.float32))
    Ws = [np.asarray(W, dtype=np.float32) for W in (Wq, Wk, Wv)]
    bs = [np.asarray(bb, dtype=np.float32) for bb in (bq, bk, bv)]

    # ---- phase A host prep ----
    xta = np.zeros((NPAD, B), np.float32)
    xta[:S] = x.T
    xta[S, :] = 1.0  # ones row folds the bias into the matmul
    xt_tiled = np.ascontiguousarray(
        xta.reshape(NBLK, 128, B).transpose(1, 0, 2).reshape(128, NBLK * B)
    ).astype(np.float16)
    in_maps_a = []
    for c in range(N_CORES):
        m = {"xt": xt_tiled}
        sl = slice(c * MSL, (c + 1) * MSL)
        for name, W, bias in zip("qkv", Ws, bs):
            wa = np.zeros((NPAD, MSL), np.float32)
            wa[:S] = W[sl].T
            wa[S] = bias[sl]
            m[f"w{name}"] = np.ascontiguousarray(
                wa.reshape(NBLK, 128, MSL).transpose(1, 0, 2).reshape(128, NBLK * MSL)
            ).astype(np.float16)
        in_maps_a.append(m)

    res_a = _run(_get("proj", _build_proj), in_maps_a, "proj")
    q = np.concatenate([res_a[c]["oq"] for c in range(N_CORES)], axis=1)
    k = np.concatenate([res_a[c]["ok"] for c in range(N_CORES)], axis=1)
    v = np.concatenate([res_a[c]["ov"] for c in range(N_CORES)], axis=1)

    # ---- phase B host prep ----
    mask = np.ascontiguousarray(
        np.triu(np.ones((128, 128))).astype(ml_dtypes.bfloat16)
    )
    in_maps_b = []
    for c in range(N_CORES):
        ex = slice(BPC * c, BPC * (c + 1))
        qb = np.ascontiguousarray(
            np.broadcast_to(q[ex][:, None, :], (BPC, 128, S))
        )
        ktc = np.ascontiguousarray(
            k[ex].reshape(BPC, NJB, 128).transpose(0, 2, 1)
        )
        vtc = v[ex].reshape(BPC, NJB, 128).transpose(0, 2, 1)
        w2 = np.empty((BPC, 128, 2 * NJB), np.float32)
        w2[:, :, 0::2] = vtc
        w2[:, :, 1::2] = 1.0
        w2 = w2.astype(ml_dtypes.bfloat16)
        in_maps_b.append({"qb": qb, "kt": ktc, "w2": w2, "mask": mask})

    res_b = _run(_get("attn", _build_attn), in_maps_b, "attn")
    out = np.concatenate([res_b[c]["out"] for c in range(N_CORES)], axis=0)
    return out
